# revision 16
# baseline (speedup 1.0000x reference)
"""VQ codebook (BooleanAnchorTable) Trainium2 kernel.

Problem: z (8,1024,512) f32 tokens, emb (8192,512) f32 codebook.
  dist[t,k] = ||z_t||^2 - 2 z_t.e_k + ||e_k||^2 ; idx = argmin_k dist
  z_st = z + (z_q - z) (straight-through), z_q = emb[idx]
  codebook_loss = commit_loss = mean((z_q - z)^2)

Sharding: data-parallel over the b*t token axis. Each of the 8 cores gets
1024 tokens (one batch row) plus the full codebook (replicated). No
collectives: the two scalar losses are reduced on the host from the
device outputs (an unshard-style reduction).

Per-core device kernel:
  score[t,k] = z_t.e_k - 0.5||e_k||^2   (argmax score == argmin dist)
  - PE: for each (tok128 x code512) tile: one K=1 matmul folds the
    -0.5||e||^2 row into PSUM (start=True), then 4 K=128 matmuls
    accumulate z.T @ embT. All fp32 (exact argmin match needs fp32).
  - ACT copies PSUM -> SBUF scores [128, 8192].
  - DVE vector.max + max_index give per-token argmax over all 8192
    codes in one shot (first-occurrence tie-break, same as argmin).
  - gpsimd indirect DMA gathers emb[idx] rows -> z_st output.
Host: z_st = z + (z_q - z) fp32 (mirrors reference rounding), losses
from (z_q - z) in fp32, idx cast to int32.
"""

import os

import numpy as np

import concourse.bass as bass
import concourse.mybir as mybir
import concourse.tile as tile
from concourse import bacc, dve_ops
from concourse.bass import IndirectOffsetOnAxis, ts
from concourse.bass_utils import run_bass_kernel_spmd

F32 = mybir.dt.float32
U32 = mybir.dt.uint32

B, T, H, K = 8, 1024, 512, 8192
N_CORES = 8
TOK = B * T // N_CORES        # 1024 tokens per core
TCH = TOK // 128              # 8 token chunks of 128
CCH = K // 512                # 16 code chunks of 512
KCH = H // 128                # 4 contraction chunks of 128

# "scan": single-pass argmax via a custom DVE op on a reversed stream
# "maxidx": stock vector.max + max_index (two passes)
ARGMAX_MODE = os.environ.get("ARGMAX_MODE", "scan")
# repeat the whole compute R times inside one NEFF (timing via differencing)
REPEATS = int(os.environ.get("BASS_VQ_REPEATS", "1"))

_PROGRAM_CACHE = {}


def _register_argmax_op():
    """Custom DVE op: one-pass argmax over a REVERSED stream.

    body[k] = Idx if x[k] == running_max(x[0..k]) else -FLT_MAX
    accum_out = max(body)  ->  position of the LAST prefix-maximum.
    Fed the reversed scores, the last prefix-max position in the reversed
    stream corresponds to the FIRST occurrence of the global max in the
    forward stream (numpy argmax/argmin tie semantics).
    idx = (N-1) - accum_out.
    """
    if "ARGMAX_REV_ANT" in dve_ops._SUB_OPCODE_FOR_NAME:
        return next(op for op in dve_ops.OPS if op.name == "ARGMAX_REV_ANT")

    from concourse.dve_spec import (
        AluOp, Idx, MaxNeg, Spec, Src0, eq, lower, scan, select,
    )
    from concourse.dve_uop import DveOpSpec

    FMAX = np.float32(np.finfo(np.float32).max)

    def _ref(in0, in1, c0, c1, c2):
        x = in0.astype(np.float32).reshape(in0.shape[0], -1)
        r = np.maximum.accumulate(x, axis=1)
        idxs = np.broadcast_to(np.arange(x.shape[1], dtype=np.float32), x.shape)
        out = np.where(x == r, idxs, -FMAX)
        return out, out.max(axis=1, keepdims=True)

    spec = Spec(
        body=select(eq(Src0, scan(AluOp.MAX, Src0)), Idx, MaxNeg),
        accum=AluOp.MAX,
        reference=_ref,
    )
    row = max(dve_ops._SUB_OPCODE_FOR_NAME.values()) + 1
    assert row < 0x20
    dve_ops._SUB_OPCODE_FOR_NAME["ARGMAX_REV_ANT"] = row
    shas = {}
    for ver in ("v3", "v4"):
        uops = lower(spec, ver=ver)
        shas[ver] = DveOpSpec(
            name="ARGMAX_REV_ANT", opcode=row, uops=uops, rd1_en=False
        ).sha(ver)
    op = dve_ops.DveOp("ARGMAX_REV_ANT", spec, subdim=False, uops_sha=shas)
    dve_ops.OPS.append(op)
    dve_ops.CUSTOM_DVE_SPECS["ARGMAX_REV_ANT"] = spec
    return op


_ARGMAX_OP = _register_argmax_op() if ARGMAX_MODE == "scan" else None


def build_program(repeats=None):
    if repeats is None:
        repeats = REPEATS
    nc = bacc.Bacc(debug=False)

    zt_d = nc.dram_tensor("zt", (KCH, 128, TOK), F32, kind="ExternalInput")
    embt_d = nc.dram_tensor("embt", (KCH, 128, K), F32, kind="ExternalInput")
    e2n_d = nc.dram_tensor("e2n", (1, K), F32, kind="ExternalInput")
    zn_d = nc.dram_tensor("zn", (128, TCH), F32, kind="ExternalInput")
    embn_d = nc.dram_tensor("embn", (K, H), F32, kind="ExternalInput")
    zst_d = nc.dram_tensor("zst", (TOK, H), F32, kind="ExternalOutput")
    idx_d = nc.dram_tensor("idx", (TCH, 128), U32, kind="ExternalOutput")

    with tile.TileContext(nc) as tc:
        with (
            tc.tile_pool(name="static", bufs=1) as static,
            tc.tile_pool(name="scoresp", bufs=2) as scoresp,
            tc.tile_pool(name="ztp", bufs=2) as ztp,
            tc.tile_pool(name="e2p", bufs=8) as e2p,
            tc.tile_pool(name="zqp", bufs=2) as zqp,
            tc.tile_pool(name="smallp", bufs=2) as smallp,
            tc.tile_pool(name="psum", bufs=6, space="PSUM") as psum,
        ):
          for _rep in range(repeats):
            # full transposed codebook resident: 128KB/partition.
            # Loaded in code-chunk-major [128,512] pieces so the first
            # matmuls can start after ~1MB instead of ~17MB.
            embt_sb = static.tile([128, KCH, K], F32, tag="embt")
            for c in range(CCH):
                for k in range(KCH):
                    nc.sync.dma_start(
                        out=embt_sb[:, k, ts(c, 512)],
                        in_=embt_d.ap()[k, :, ts(c, 512)],
                    )
            zn_sb = static.tile([128, TCH], F32, tag="zn")
            nc.sync.dma_start(out=zn_sb, in_=zn_d.ap())
            ones_sb = static.tile([1, 128], F32, tag="ones")
            nc.vector.memset(ones_sb, 1.0)

            for t in range(TCH):
                zt_sb = ztp.tile([128, KCH, 128], F32, tag="zt")
                for k in range(KCH):
                    nc.sync.dma_start(
                        out=zt_sb[:, k], in_=zt_d.ap()[k, :, ts(t, 128)]
                    )
                scores = scoresp.tile([128, K], F32, tag="scores")
                for c in range(CCH):
                    e2c = e2p.tile([1, 512], F32, tag="e2c")
                    nc.sync.dma_start(out=e2c, in_=e2n_d.ap()[:, ts(c, 512)])
                    ps = psum.tile([128, 512], F32, tag="ps")
                    # psum := -||e||^2 broadcast to all 128 partitions,
                    # then += (2z).e ; scores := psum - ||z_t||^2 (one final
                    # fp32 rounding at ~dist magnitude: matches the
                    # reference's coarse quantization so near-ties tie and
                    # resolve by first-index, like jnp.argmin)
                    nc.tensor.matmul(ps, ones_sb, e2c, start=True, stop=False)
                    for k in range(KCH):
                        nc.tensor.matmul(
                            ps,
                            zt_sb[:, k],
                            embt_sb[:, k, ts(c, 512)],
                            start=False,
                            stop=(k == KCH - 1),
                        )
                    nc.scalar.activation(
                        scores[:, ts(c, 512)], ps,
                        mybir.ActivationFunctionType.Identity,
                        bias=zn_sb[:, t:t + 1], scale=1.0,
                    )

                if ARGMAX_MODE == "scan":
                    ridx = smallp.tile([128, 1], F32, tag="ridx")
                    rev = scores[:, ::-1]
                    nc.vector._custom_dve(
                        _ARGMAX_OP, out=rev, in0=rev, accum_out=ridx
                    )
                    gidx = smallp.tile([128, 1], U32, tag="gidx")
                    # idx = (K-1) - ridx, cast f32 -> u32 on write
                    nc.vector.tensor_scalar(
                        out=gidx, in0=ridx, scalar1=-1.0, scalar2=float(K - 1),
                        op0=mybir.AluOpType.mult, op1=mybir.AluOpType.add,
                    )
                    gidx_col = gidx
                else:
                    gmax = smallp.tile([128, 8], F32, tag="gmax")
                    nc.vector.max(out=gmax, in_=scores)
                    gidx = smallp.tile([128, 8], U32, tag="gidx")
                    nc.vector.max_index(out=gidx, in_max=gmax, in_values=scores)
                    gidx_col = gidx[:, 0:1]
                nc.sync.dma_start(out=idx_d.ap()[t], in_=gidx_col)

                zq = zqp.tile([128, H], F32, tag="zq")
                nc.gpsimd.indirect_dma_start(
                    out=zq,
                    out_offset=None,
                    in_=embn_d.ap(),
                    in_offset=IndirectOffsetOnAxis(ap=gidx_col, axis=0),
                )
                nc.sync.dma_start(out=zst_d.ap()[ts(t, 128)], in_=zq)
    nc.compile()
    return nc


def _get_program(repeats=None):
    if repeats is None:
        repeats = REPEATS
    if repeats not in _PROGRAM_CACHE:
        _PROGRAM_CACHE[repeats] = build_program(repeats)
    return _PROGRAM_CACHE[repeats]


def kernel(z, emb):
    z = np.ascontiguousarray(np.asarray(z, dtype=np.float32))
    emb = np.ascontiguousarray(np.asarray(emb, dtype=np.float32))

    embt = np.ascontiguousarray(emb.T * np.float32(2.0)).reshape(KCH, 128, K)
    e2n = (-np.sum(emb * emb, axis=1, dtype=np.float32)).reshape(1, K)

    in_maps = []
    for c in range(N_CORES):
        ztc = np.ascontiguousarray(z[c].T).reshape(KCH, 128, TOK)
        zn = -np.sum(z[c] * z[c], axis=1, dtype=np.float32)
        zn = np.ascontiguousarray(zn.reshape(TCH, 128).T)
        in_maps.append(
            {"zt": ztc, "embt": embt, "e2n": e2n, "zn": zn, "embn": emb}
        )

    nc = _get_program()
    res = run_bass_kernel_spmd(nc, in_maps, core_ids=list(range(N_CORES)))

    z_q = np.stack([r["zst"] for r in res.results]).reshape(B, T, H)
    idx = np.stack([r["idx"].reshape(-1) for r in res.results])
    idx = idx.astype(np.int32).reshape(B, T)

    # straight-through output with the same fp32 rounding as the reference
    z_st = z + (z_q - z)
    diff = z_q - z
    loss = np.float32(np.mean(diff * diff, dtype=np.float32))
    return z_st, idx, loss, loss


# revision 18
# speedup vs baseline: 94.0721x; 94.0721x over previous
"""VQ codebook (BooleanAnchorTable) Trainium2 kernel.

Problem: z (8,1024,512) f32 tokens, emb (8192,512) f32 codebook.
  dist[t,k] = ||z_t||^2 - 2 z_t.e_k + ||e_k||^2 ; idx = argmin_k dist
  z_st = z + (z_q - z) (straight-through), z_q = emb[idx]
  codebook_loss = commit_loss = mean((z_q - z)^2)

Sharding: data-parallel over the b*t token axis. Each of the 8 cores gets
1024 tokens (one batch row) plus the full codebook (replicated). No
collectives: the two scalar losses are reduced on the host from the
device outputs (an unshard-style reduction).

Per-core device kernel:
  score[t,k] = z_t.e_k - 0.5||e_k||^2   (argmax score == argmin dist)
  - PE: for each (tok128 x code512) tile: one K=1 matmul folds the
    -0.5||e||^2 row into PSUM (start=True), then 4 K=128 matmuls
    accumulate z.T @ embT. All fp32 (exact argmin match needs fp32).
  - ACT copies PSUM -> SBUF scores [128, 8192].
  - DVE vector.max + max_index give per-token argmax over all 8192
    codes in one shot (first-occurrence tie-break, same as argmin).
  - gpsimd indirect DMA gathers emb[idx] rows -> z_st output.
Host: z_st = z + (z_q - z) fp32 (mirrors reference rounding), losses
from (z_q - z) in fp32, idx cast to int32.
"""

import os

import numpy as np

import concourse.bass as bass
import concourse.mybir as mybir
import concourse.tile as tile
from concourse import bacc, dve_ops
from concourse.bass import IndirectOffsetOnAxis, ts
from concourse.bass_utils import run_bass_kernel_spmd

F32 = mybir.dt.float32
U32 = mybir.dt.uint32

B, T, H, K = 8, 1024, 512, 8192
N_CORES = 8
TOK = B * T // N_CORES        # 1024 tokens per core
TCH = TOK // 128              # 8 token chunks of 128
CCH = K // 512                # 16 code chunks of 512
KCH = H // 128                # 4 contraction chunks of 128

# "scan": single-pass argmax via a custom DVE op on a reversed stream
# "maxidx": stock vector.max + max_index (two passes)
ARGMAX_MODE = os.environ.get("ARGMAX_MODE", "scan")
# repeat the whole compute R times inside one NEFF (timing via differencing)
REPEATS = int(os.environ.get("BASS_VQ_REPEATS", "1"))

_PROGRAM_CACHE = {}


def _register_argmax_op():
    """Custom DVE op: one-pass argmax over a REVERSED stream.

    body[k] = Idx if x[k] == running_max(x[0..k]) else -FLT_MAX
    accum_out = max(body)  ->  position of the LAST prefix-maximum.
    Fed the reversed scores, the last prefix-max position in the reversed
    stream corresponds to the FIRST occurrence of the global max in the
    forward stream (numpy argmax/argmin tie semantics).
    idx = (N-1) - accum_out.
    """
    if "ARGMAX_REV_ANT" in dve_ops._SUB_OPCODE_FOR_NAME:
        return next(op for op in dve_ops.OPS if op.name == "ARGMAX_REV_ANT")

    from concourse.dve_spec import (
        AluOp, Idx, MaxNeg, Spec, Src0, eq, lower, scan, select,
    )
    from concourse.dve_uop import DveOpSpec

    FMAX = np.float32(np.finfo(np.float32).max)

    def _ref(in0, in1, c0, c1, c2):
        x = in0.astype(np.float32).reshape(in0.shape[0], -1)
        r = np.maximum.accumulate(x, axis=1)
        idxs = np.broadcast_to(np.arange(x.shape[1], dtype=np.float32), x.shape)
        out = np.where(x == r, idxs, -FMAX)
        return out, out.max(axis=1, keepdims=True)

    spec = Spec(
        body=select(eq(Src0, scan(AluOp.MAX, Src0)), Idx, MaxNeg),
        accum=AluOp.MAX,
        reference=_ref,
    )
    row = max(dve_ops._SUB_OPCODE_FOR_NAME.values()) + 1
    assert row < 0x20
    dve_ops._SUB_OPCODE_FOR_NAME["ARGMAX_REV_ANT"] = row
    shas = {}
    for ver in ("v3", "v4"):
        uops = lower(spec, ver=ver)
        shas[ver] = DveOpSpec(
            name="ARGMAX_REV_ANT", opcode=row, uops=uops, rd1_en=False
        ).sha(ver)
    op = dve_ops.DveOp("ARGMAX_REV_ANT", spec, subdim=False, uops_sha=shas)
    dve_ops.OPS.append(op)
    dve_ops.CUSTOM_DVE_SPECS["ARGMAX_REV_ANT"] = spec
    return op


_ARGMAX_OP = _register_argmax_op() if ARGMAX_MODE == "scan" else None


def build_program(repeats=None):
    if repeats is None:
        repeats = REPEATS
    nc = bacc.Bacc(debug=False)

    zt_d = nc.dram_tensor("zt", (KCH, 128, TOK), F32, kind="ExternalInput")
    embt_d = nc.dram_tensor("embt", (KCH, 128, K), F32, kind="ExternalInput")
    e2n_d = nc.dram_tensor("e2n", (1, K), F32, kind="ExternalInput")
    zn_d = nc.dram_tensor("zn", (128, TCH), F32, kind="ExternalInput")
    embn_d = nc.dram_tensor("embn", (K, H), F32, kind="ExternalInput")
    zst_d = nc.dram_tensor("zst", (TOK, H), F32, kind="ExternalOutput")
    idx_d = nc.dram_tensor("idx", (TCH, 128), U32, kind="ExternalOutput")

    with tile.TileContext(nc) as tc:
        with (
            tc.tile_pool(name="static", bufs=1) as static,
            tc.tile_pool(name="scoresp", bufs=2) as scoresp,
            tc.tile_pool(name="ztp", bufs=2) as ztp,
            tc.tile_pool(name="e2p", bufs=3) as e2p,
            tc.tile_pool(name="zqp", bufs=2) as zqp,
            tc.tile_pool(name="smallp", bufs=2) as smallp,
            tc.tile_pool(name="psum", bufs=6, space="PSUM") as psum,
        ):
          for _rep in range(repeats):
            # full transposed codebook resident: 128KB/partition.
            # Loaded in code-chunk-major [128,512] pieces so the first
            # matmuls can start after ~1MB instead of ~17MB.
            embt_sb = static.tile([128, KCH, K], F32, tag="embt")
            for c in range(CCH):
                for k in range(KCH):
                    nc.sync.dma_start(
                        out=embt_sb[:, k, ts(c, 512)],
                        in_=embt_d.ap()[k, :, ts(c, 512)],
                    )
            zn_sb = static.tile([128, TCH], F32, tag="zn")
            nc.sync.dma_start(out=zn_sb, in_=zn_d.ap())
            ones_sb = static.tile([1, 128], F32, tag="ones")
            nc.vector.memset(ones_sb, 1.0)

            for t in range(TCH):
                zt_sb = ztp.tile([128, KCH, 128], F32, tag="zt")
                for k in range(KCH):
                    nc.sync.dma_start(
                        out=zt_sb[:, k], in_=zt_d.ap()[k, :, ts(t, 128)]
                    )
                scores = scoresp.tile([128, K], F32, tag="scores")
                for c in range(CCH):
                    e2c = e2p.tile([1, 512], F32, tag="e2c")
                    nc.sync.dma_start(out=e2c, in_=e2n_d.ap()[:, ts(c, 512)])
                    ps = psum.tile([128, 512], F32, tag="ps")
                    # psum := -||e||^2 broadcast to all 128 partitions,
                    # then += (2z).e ; scores := psum - ||z_t||^2 (one final
                    # fp32 rounding at ~dist magnitude: matches the
                    # reference's coarse quantization so near-ties tie and
                    # resolve by first-index, like jnp.argmin)
                    nc.tensor.matmul(ps, ones_sb, e2c, start=True, stop=False)
                    for k in range(KCH):
                        nc.tensor.matmul(
                            ps,
                            zt_sb[:, k],
                            embt_sb[:, k, ts(c, 512)],
                            start=False,
                            stop=(k == KCH - 1),
                        )
                    nc.scalar.activation(
                        scores[:, ts(c, 512)], ps,
                        mybir.ActivationFunctionType.Identity,
                        bias=zn_sb[:, t:t + 1], scale=1.0,
                    )

                if ARGMAX_MODE == "scan":
                    ridx = smallp.tile([128, 1], F32, tag="ridx")
                    rev = scores[:, ::-1]
                    nc.vector._custom_dve(
                        _ARGMAX_OP, out=rev, in0=rev, accum_out=ridx
                    )
                    gidx = smallp.tile([128, 1], U32, tag="gidx")
                    # idx = (K-1) - ridx, cast f32 -> u32 on write
                    nc.vector.tensor_scalar(
                        out=gidx, in0=ridx, scalar1=-1.0, scalar2=float(K - 1),
                        op0=mybir.AluOpType.mult, op1=mybir.AluOpType.add,
                    )
                    gidx_col = gidx
                else:
                    gmax = smallp.tile([128, 8], F32, tag="gmax")
                    nc.vector.max(out=gmax, in_=scores)
                    gidx = smallp.tile([128, 8], U32, tag="gidx")
                    nc.vector.max_index(out=gidx, in_max=gmax, in_values=scores)
                    gidx_col = gidx[:, 0:1]
                nc.sync.dma_start(out=idx_d.ap()[t], in_=gidx_col)

                zq = zqp.tile([128, H], F32, tag="zq")
                nc.gpsimd.indirect_dma_start(
                    out=zq,
                    out_offset=None,
                    in_=embn_d.ap(),
                    in_offset=IndirectOffsetOnAxis(ap=gidx_col, axis=0),
                )
                nc.sync.dma_start(out=zst_d.ap()[ts(t, 128)], in_=zq)
    nc.compile()
    return nc


def _get_program(repeats=None):
    if repeats is None:
        repeats = REPEATS
    if repeats not in _PROGRAM_CACHE:
        _PROGRAM_CACHE[repeats] = build_program(repeats)
    return _PROGRAM_CACHE[repeats]


def kernel(z, emb):
    z = np.ascontiguousarray(np.asarray(z, dtype=np.float32))
    emb = np.ascontiguousarray(np.asarray(emb, dtype=np.float32))

    embt = np.ascontiguousarray(emb.T * np.float32(2.0)).reshape(KCH, 128, K)
    e2n = (-np.sum(emb * emb, axis=1, dtype=np.float32)).reshape(1, K)

    in_maps = []
    for c in range(N_CORES):
        ztc = np.ascontiguousarray(z[c].T).reshape(KCH, 128, TOK)
        zn = -np.sum(z[c] * z[c], axis=1, dtype=np.float32)
        zn = np.ascontiguousarray(zn.reshape(TCH, 128).T)
        in_maps.append(
            {"zt": ztc, "embt": embt, "e2n": e2n, "zn": zn, "embn": emb}
        )

    nc = _get_program()
    res = run_bass_kernel_spmd(nc, in_maps, core_ids=list(range(N_CORES)))

    z_q = np.stack([r["zst"] for r in res.results]).reshape(B, T, H)
    idx = np.stack([r["idx"].reshape(-1) for r in res.results])
    idx = idx.astype(np.int32).reshape(B, T)

    # straight-through output with the same fp32 rounding as the reference
    z_st = z + (z_q - z)
    diff = z_q - z
    loss = np.float32(np.mean(diff * diff, dtype=np.float32))
    return z_st, idx, loss, loss


# revision 21
# speedup vs baseline: 103.3617x; 1.0987x over previous
"""VQ codebook (BooleanAnchorTable) Trainium2 kernel.

Problem: z (8,1024,512) f32 tokens, emb (8192,512) f32 codebook.
  dist[t,k] = ||z_t||^2 - 2 z_t.e_k + ||e_k||^2 ; idx = argmin_k dist
  z_st = z + (z_q - z) (straight-through), z_q = emb[idx]
  codebook_loss = commit_loss = mean((z_q - z)^2)

Sharding: data-parallel over the b*t token axis. Each of the 8 cores gets
1024 tokens (one batch row) plus the full codebook (replicated). No
collectives: the two scalar losses are reduced on the host from the
device outputs (an unshard-style reduction).

Per-core device kernel:
  score[t,k] = z_t.e_k - 0.5||e_k||^2   (argmax score == argmin dist)
  - PE: for each (tok128 x code512) tile: one K=1 matmul folds the
    -0.5||e||^2 row into PSUM (start=True), then 4 K=128 matmuls
    accumulate z.T @ embT. All fp32 (exact argmin match needs fp32).
  - ACT copies PSUM -> SBUF scores [128, 8192].
  - DVE vector.max + max_index give per-token argmax over all 8192
    codes in one shot (first-occurrence tie-break, same as argmin).
  - gpsimd indirect DMA gathers emb[idx] rows -> z_st output.
Host: z_st = z + (z_q - z) fp32 (mirrors reference rounding), losses
from (z_q - z) in fp32, idx cast to int32.
"""

import os

import numpy as np

import concourse.bass as bass
import concourse.mybir as mybir
import concourse.tile as tile
from concourse import bacc, dve_ops
from concourse.bass import IndirectOffsetOnAxis, ts
from concourse.bass_utils import run_bass_kernel_spmd

F32 = mybir.dt.float32
U32 = mybir.dt.uint32

B, T, H, K = 8, 1024, 512, 8192
N_CORES = 8
TOK = B * T // N_CORES        # 1024 tokens per core
TCH = TOK // 128              # 8 token chunks of 128
CCH = K // 512                # 16 code chunks of 512
KCH = H // 128                # 4 contraction chunks of 128

# "scan": single-pass argmax via a custom DVE op on a reversed stream
# "maxidx": stock vector.max + max_index (two passes)
ARGMAX_MODE = os.environ.get("ARGMAX_MODE", "scan")
# repeat the whole compute R times inside one NEFF (timing via differencing)
REPEATS = int(os.environ.get("BASS_VQ_REPEATS", "1"))

_PROGRAM_CACHE = {}


def _register_argmax_op():
    """Custom DVE op: one-pass argmax over a REVERSED stream.

    body[k] = Idx if x[k] == running_max(x[0..k]) else -FLT_MAX
    accum_out = max(body)  ->  position of the LAST prefix-maximum.
    Fed the reversed scores, the last prefix-max position in the reversed
    stream corresponds to the FIRST occurrence of the global max in the
    forward stream (numpy argmax/argmin tie semantics).
    idx = (N-1) - accum_out.
    """
    if "ARGMAX_REV_ANT" in dve_ops._SUB_OPCODE_FOR_NAME:
        return next(op for op in dve_ops.OPS if op.name == "ARGMAX_REV_ANT")

    from concourse.dve_spec import (
        AluOp, Idx, MaxNeg, Spec, Src0, eq, lower, scan, select,
    )
    from concourse.dve_uop import DveOpSpec

    FMAX = np.float32(np.finfo(np.float32).max)

    def _ref(in0, in1, c0, c1, c2):
        x = in0.astype(np.float32).reshape(in0.shape[0], -1)
        r = np.maximum.accumulate(x, axis=1)
        idxs = np.broadcast_to(np.arange(x.shape[1], dtype=np.float32), x.shape)
        out = np.where(x == r, idxs, -FMAX)
        return out, out.max(axis=1, keepdims=True)

    spec = Spec(
        body=select(eq(Src0, scan(AluOp.MAX, Src0)), Idx, MaxNeg),
        accum=AluOp.MAX,
        reference=_ref,
    )
    row = max(dve_ops._SUB_OPCODE_FOR_NAME.values()) + 1
    assert row < 0x20
    dve_ops._SUB_OPCODE_FOR_NAME["ARGMAX_REV_ANT"] = row
    shas = {}
    for ver in ("v3", "v4"):
        uops = lower(spec, ver=ver)
        shas[ver] = DveOpSpec(
            name="ARGMAX_REV_ANT", opcode=row, uops=uops, rd1_en=False
        ).sha(ver)
    op = dve_ops.DveOp("ARGMAX_REV_ANT", spec, subdim=False, uops_sha=shas)
    dve_ops.OPS.append(op)
    dve_ops.CUSTOM_DVE_SPECS["ARGMAX_REV_ANT"] = spec
    return op


_ARGMAX_OP = _register_argmax_op() if ARGMAX_MODE == "scan" else None


def build_program(repeats=None, tiny_io=False):
    if repeats is None:
        repeats = REPEATS
    nc = bacc.Bacc(debug=False)

    zt_d = nc.dram_tensor("zt", (KCH, 128, TOK), F32, kind="ExternalInput")
    embt_d = nc.dram_tensor("embt", (KCH, 128, K), F32, kind="ExternalInput")
    e2n_d = nc.dram_tensor("e2n", (1, K), F32, kind="ExternalInput")
    zn_d = nc.dram_tensor("zn", (128, TCH), F32, kind="ExternalInput")
    embn_d = nc.dram_tensor("embn", (K, H), F32, kind="ExternalInput")
    if tiny_io:
        # timing-only variant: keep the zq gather live but write just one
        # column so host<->device transfer noise doesn't swamp timing
        zst_d = nc.dram_tensor("zst", (TCH, 128), F32, kind="ExternalOutput")
    else:
        zst_d = nc.dram_tensor("zst", (TOK, H), F32, kind="ExternalOutput")
    idx_d = nc.dram_tensor("idx", (TCH, 128), U32, kind="ExternalOutput")

    with tile.TileContext(nc) as tc:
        with (
            tc.tile_pool(name="static", bufs=1) as static,
            tc.tile_pool(name="scoresp", bufs=2) as scoresp,
            tc.tile_pool(name="ztp", bufs=2) as ztp,
            tc.tile_pool(name="e2p", bufs=3) as e2p,
            tc.tile_pool(name="zqp", bufs=2) as zqp,
            tc.tile_pool(name="smallp", bufs=2) as smallp,
            tc.tile_pool(name="psum", bufs=6, space="PSUM") as psum,
        ):
          for _rep in range(repeats):
            # full transposed codebook resident: 128KB/partition.
            # Loaded in code-chunk-major [128,512] pieces so the first
            # matmuls can start after ~1MB instead of ~17MB.
            embt_sb = static.tile([128, KCH, K], F32, tag="embt")
            for c in range(CCH):
                for k in range(KCH):
                    nc.sync.dma_start(
                        out=embt_sb[:, k, ts(c, 512)],
                        in_=embt_d.ap()[k, :, ts(c, 512)],
                    )
            zn_sb = static.tile([128, TCH], F32, tag="zn")
            nc.sync.dma_start(out=zn_sb, in_=zn_d.ap())
            ones_sb = static.tile([1, 128], F32, tag="ones")
            nc.vector.memset(ones_sb, 1.0)

            for t in range(TCH):
                zt_sb = ztp.tile([128, KCH, 128], F32, tag="zt")
                for k in range(KCH):
                    nc.sync.dma_start(
                        out=zt_sb[:, k], in_=zt_d.ap()[k, :, ts(t, 128)]
                    )
                scores = scoresp.tile([128, K], F32, tag="scores")
                for c in range(CCH):
                    e2c = e2p.tile([1, 512], F32, tag="e2c")
                    nc.sync.dma_start(out=e2c, in_=e2n_d.ap()[:, ts(c, 512)])
                    ps = psum.tile([128, 512], F32, tag="ps")
                    # psum := -||e||^2 broadcast to all 128 partitions,
                    # then += (2z).e ; scores := psum - ||z_t||^2 (one final
                    # fp32 rounding at ~dist magnitude: matches the
                    # reference's coarse quantization so near-ties tie and
                    # resolve by first-index, like jnp.argmin)
                    nc.tensor.matmul(ps, ones_sb, e2c, start=True, stop=False)
                    for k in range(KCH):
                        nc.tensor.matmul(
                            ps,
                            zt_sb[:, k],
                            embt_sb[:, k, ts(c, 512)],
                            start=False,
                            stop=(k == KCH - 1),
                        )
                    nc.scalar.activation(
                        scores[:, ts(c, 512)], ps,
                        mybir.ActivationFunctionType.Identity,
                        bias=zn_sb[:, t:t + 1], scale=1.0,
                    )

                if ARGMAX_MODE == "scan":
                    ridx = smallp.tile([128, 1], F32, tag="ridx")
                    rev = scores[:, ::-1]
                    nc.vector._custom_dve(
                        _ARGMAX_OP, out=rev, in0=rev, accum_out=ridx
                    )
                    gidx = smallp.tile([128, 1], U32, tag="gidx")
                    # idx = (K-1) - ridx, cast f32 -> u32 on write
                    nc.vector.tensor_scalar(
                        out=gidx, in0=ridx, scalar1=-1.0, scalar2=float(K - 1),
                        op0=mybir.AluOpType.mult, op1=mybir.AluOpType.add,
                    )
                    gidx_col = gidx
                else:
                    gmax = smallp.tile([128, 8], F32, tag="gmax")
                    nc.vector.max(out=gmax, in_=scores)
                    gidx = smallp.tile([128, 8], U32, tag="gidx")
                    nc.vector.max_index(out=gidx, in_max=gmax, in_values=scores)
                    gidx_col = gidx[:, 0:1]
                nc.sync.dma_start(out=idx_d.ap()[t], in_=gidx_col)

                zq = zqp.tile([128, H], F32, tag="zq")
                nc.gpsimd.indirect_dma_start(
                    out=zq,
                    out_offset=None,
                    in_=embn_d.ap(),
                    in_offset=IndirectOffsetOnAxis(ap=gidx_col, axis=0),
                )
                if tiny_io:
                    nc.sync.dma_start(out=zst_d.ap()[t], in_=zq[:, 0:1])
                else:
                    nc.sync.dma_start(out=zst_d.ap()[ts(t, 128)], in_=zq)
    nc.compile()
    return nc


def _get_program(repeats=None, tiny_io=False):
    if repeats is None:
        repeats = REPEATS
    key = (repeats, tiny_io)
    if key not in _PROGRAM_CACHE:
        _PROGRAM_CACHE[key] = build_program(repeats, tiny_io)
    return _PROGRAM_CACHE[key]


def kernel(z, emb):
    z = np.ascontiguousarray(np.asarray(z, dtype=np.float32))
    emb = np.ascontiguousarray(np.asarray(emb, dtype=np.float32))

    embt = np.ascontiguousarray(emb.T * np.float32(2.0)).reshape(KCH, 128, K)
    e2n = (-np.sum(emb * emb, axis=1, dtype=np.float32)).reshape(1, K)

    in_maps = []
    for c in range(N_CORES):
        ztc = np.ascontiguousarray(z[c].T).reshape(KCH, 128, TOK)
        zn = -np.sum(z[c] * z[c], axis=1, dtype=np.float32)
        zn = np.ascontiguousarray(zn.reshape(TCH, 128).T)
        in_maps.append(
            {"zt": ztc, "embt": embt, "e2n": e2n, "zn": zn, "embn": emb}
        )

    nc = _get_program()
    res = run_bass_kernel_spmd(nc, in_maps, core_ids=list(range(N_CORES)))

    z_q = np.stack([r["zst"] for r in res.results]).reshape(B, T, H)
    idx = np.stack([r["idx"].reshape(-1) for r in res.results])
    idx = idx.astype(np.int32).reshape(B, T)

    # straight-through output with the same fp32 rounding as the reference
    z_st = z + (z_q - z)
    diff = z_q - z
    loss = np.float32(np.mean(diff * diff, dtype=np.float32))
    return z_st, idx, loss, loss
